# revision 59
# baseline (speedup 1.0000x reference)
"""Trainium2 Bass kernel for nn_DLFG_79817672229311 (segment_reduce).

Computes, data-parallel over the batch axis on 8 NeuronCores:
  history = [extInfo, ratings, 1]                    # [BS, 20033] per core
  x1 = lrelu(history @ [w1;b1].T); BN folded into w2 on host
  x2..x5 = lrelu(x @ wl.T + bl)
  gen = tanh(x5 @ w6.T + b6)                         # [BS, 65]
  s, cnt = per-row sum / count of nonzero ratings
  addv = s/cnt + gen[:, 64]
  out = gen[:, :64] @ movie_factors.T + addv[:, None] + movie_bias

Design (per core; layer 1 is at the fp8 DoubleRow compute wall ~135us, so
everything else hides under or packs tightly around it):
- Activations ride transposed ([feature, batch]): batch (512) is the matmul
  free dim, features the partition dim, so no on-device transposes are needed.
- History is staged to SBUF once in fp8 (ratings 0..5 are exact in e4m3) with
  a host-appended ones-row; layer 1 runs fp8 DoubleRow against 2^15-pre-scaled
  fp8 [w1;b1] slabs streamed from HBM, so the bias rides the contraction and
  the epilogue is a bias-free paired Lrelu (2 fo-tiles per ACT op, adjacent
  PSUM banks).
- cnt is a paired fused min+add chain on the otherwise-idle Vector engine
  under the layer-1 shadow (GpSimd was tried and poisons SBUF bandwidth).
  s is a PE ones-matmul chain over the resident history, double-buffered
  across two PSUM banks and emission-chunked into the layer-boundary gaps of
  the mid layers so it fills PE idle instead of serializing; meanV combines
  in row layout [1, BS] and is pre-scaled by the output quant scale.
- Mid layers interleave their k-accumulation across 3 PSUM banks (avoids
  same-bank drain serialization); some lrelu epilogues run on the DVE.
- addv bounces through DRAM into per-partition [128, NBT] and enters the
  reconstruction staging op as its per-partition bias, so reconstruction
  matmuls (genext bf16 x mft bf16, movie_bias via a ones-row of genext)
  start right after tanh.
- Reconstruction streams 2-bank PSUM pairs -> fused scale+bias staging on
  alternating Vector/Scalar engines (11:9) -> int8 DRAM in 2048-col DMAs
  (host dequantizes by the fixed scale 4/127).  Bias/const DMAs dispatch on
  the Scalar queue; the Sync queue carries the ht/w1 stream, late-emitted
  w2-6/mft, and the output drains.
"""

import math
import sys

sys.path.insert(0, "/opt/trn_rl_repo")

import numpy as np
import ml_dtypes

BF16 = ml_dtypes.bfloat16
FP8 = ml_dtypes.float8_e4m3

NCORES = 8
BN_EPS = 0.05
SLOPE = 0.01

FULL_CFG = dict(
    BS=512,  # per-core batch
    UINFO=32,
    M=20000,
    F=64,
    DIMS=(1024, 512, 256, 512, 1024, 65),  # fan-outs of the 6 linear layers
    HTC=4,  # history K-tiles per DMA chunk (must be even for DoubleRow pairs)
    W1_SCALE=2.0**15,  # fp8 pre-scale: w1 ~ U(+-0.007) sits in e4m3 subnormals
    MLP_SCALES=(4096.0, 4096.0, 2048.0, 4096.0),  # 2^k per layer, |w|*s < 240
    OUT_DT="i8",  # "i8" (host dequant) or "bf16"
    OUT_SCALE=127.0 / 4.0,  # int8 quantization scale (|out| <= ~3.2)
    FLIP_L1=False,  # layer 1: history stationary / w1 moving (halves LDWEIGHTS)
)


def _derived(cfg):
    d = dict(cfg)
    d["KH"] = cfg["UINFO"] + cfg["M"] + 1  # +1 ones-row carrying b1
    d["T1"] = math.ceil(d["KH"] / 128)  # history K tiles (padded)
    d["NBT"] = cfg["BS"] // 128  # batch tiles per core
    d["CHUNKS"] = [(o, min(512, cfg["M"] - o)) for o in range(0, cfg["M"], 512)]
    return d


def build_nc(cfg):
    """Build + compile the (single-core SPMD) Bass program."""
    import concourse.bass as bass
    import concourse.tile as tile
    from concourse import bacc, mybir

    d = _derived(cfg)
    BS, UINFO, M, F = cfg["BS"], cfg["UINFO"], cfg["M"], cfg["F"]
    DIMS = cfg["DIMS"]
    T1, NBT, CHUNKS, HTC = d["T1"], d["NBT"], d["CHUNKS"], cfg["HTC"]
    FO1 = DIMS[0]
    FO1T = FO1 // 128
    w1_unscale = 1.0 / cfg["W1_SCALE"]
    f32 = mybir.dt.float32
    bf16 = mybir.dt.bfloat16
    f8 = mybir.dt.float8e4
    i8 = mybir.dt.int8
    AF = mybir.ActivationFunctionType
    ALU = mybir.AluOpType

    OUT_I8 = cfg["OUT_DT"] == "i8"
    odt = i8 if OUT_I8 else bf16
    OSC = cfg["OUT_SCALE"] if OUT_I8 else 1.0

    nc = bacc.Bacc("TRN2", target_bir_lowering=False, debug=False)

    # ---- DRAM I/O ----
    ht_d = nc.dram_tensor("ht", [128, T1, BS], f8, kind="ExternalInput")
    w1t_d = nc.dram_tensor("w1t", [128, T1, FO1], f8, kind="ExternalInput")
    w_d = {}
    for li in range(2, 7):
        fi, fo = DIMS[li - 2], DIMS[li - 1]
        wdt = f8 if li < 6 else bf16
        w_d[li] = nc.dram_tensor(f"w{li}t", [128, fi // 128, fo], wdt, kind="ExternalInput")
    bp_d = {}
    for li in range(2, 6):
        fot = math.ceil(DIMS[li - 1] / 128)
        bp_d[li] = nc.dram_tensor(f"b{li}p", [128, fot], f32, kind="ExternalInput")
    b6_d = nc.dram_tensor("b6p", [128, 1], f32, kind="ExternalInput")
    mft_d = nc.dram_tensor("mft", [128, M], bf16, kind="ExternalInput")
    if cfg.get("FLIP_L1"):
        ident_d = nc.dram_tensor("ident", [128, 128], bf16, kind="ExternalInput")
    out_d = nc.dram_tensor("out", [BS, M], odt, kind="ExternalOutput")
    av_d = nc.dram_tensor("av_scr", [BS], f32)  # addv row->partition bounce

    with tile.TileContext(nc) as tc, bass.ExitStack() as ctx:
        const = ctx.enter_context(tc.tile_pool(name="const", bufs=1))
        htp = ctx.enter_context(tc.tile_pool(name="htp", bufs=1))
        w1p = ctx.enter_context(tc.tile_pool(name="w1p", bufs=8))
        actp = ctx.enter_context(tc.tile_pool(name="actp", bufs=1))
        scr = ctx.enter_context(tc.tile_pool(name="scr", bufs=2))
        ost = ctx.enter_context(tc.tile_pool(name="ost", bufs=6))
        psp = ctx.enter_context(tc.tile_pool(name="psp", bufs=4, space="PSUM"))

        # ---- constants in SBUF (dispatched on the Scalar DMA queue) ----
        bp_sb = {}
        for li in range(2, 6):
            fot = math.ceil(DIMS[li - 1] / 128)
            bp_sb[li] = const.tile([128, fot], f32, name=f"b{li}p", tag=f"b{li}p")
            nc.scalar.dma_start(out=bp_sb[li][:], in_=bp_d[li][:])
        b6_sb = const.tile([128, 1], f32, name="b6p", tag="b6p")
        nc.scalar.dma_start(out=b6_sb[:], in_=b6_d[:])
        # fp8 ones for the PE s-reduce chain, [128,2,16] so the DoubleRow
        # weight pair-step (16) meets the ISA %16 rule.  ones16m zeroes
        # history tile 0's extInfo rows; ones_last zeroes the pad partitions
        # of the final tile (incl. the bias ones-row).
        ones16 = const.tile([128, 2, 16], f8, name="ones16", tag="ones16")
        nc.vector.memset(ones16[:], 1.0)
        ones16m = const.tile([128, 2, 16], f8, name="ones16m", tag="ones16m")
        nc.vector.memset(ones16m[:], 1.0)
        nc.vector.memset(ones16m[0:UINFO, 0:1, :], 0.0)
        lastp = (UINFO + M) % 128  # partition of the bias ones-row in tile T1-1
        ones_last = const.tile([128, 2, 16], f8, name="ones_last", tag="ones_last")
        nc.vector.memset(ones_last[:], 1.0)
        nc.vector.memset(ones_last[lastp:128, :, :], 0.0)
        onesf = const.tile([128, 1], f32, name="onesf", tag="onesf")
        nc.vector.memset(onesf[:], 1.0)
        # per-partition mask for history tile 0 (extInfo rows excluded from cnt)
        rmask = const.tile([128, 1], f32, name="rmask", tag="rmask")
        nc.vector.memset(rmask[:], 1.0)
        nc.vector.memset(rmask[0:UINFO, :], 0.0)
        FLIP = cfg.get("FLIP_L1")
        if FLIP:
            ident_sb = const.tile([128, 128], bf16, name="ident", tag="ident")
            nc.scalar.dma_start(out=ident_sb[:], in_=ident_d[:])

        # ---- layer 1: one pass over history segments ----
        segs = []
        t0 = 0
        for tn in [2, 2]:
            segs.append((t0, tn))
            t0 += tn
        while t0 < T1:
            tn = min(HTC, T1 - t0)
            segs.append((t0, tn))
            t0 += tn
        NSEG = len(segs)

        x1t = actp.tile([128, FO1T, BS], f8, name="x1t", tag="x1t")
        c_acc = const.tile([128, 2, BS], f32, name="c_acc", tag="c_acc")
        # 4 paired PSUM tiles (2 banks each) -> bias-free paired epilogues
        ps1 = [psp.tile([128, 2, BS], f32, name="ps1", tag="ps") for _ in range(FO1T // 2)]

        nstep = T1 // 2 + (T1 % 2)
        ht_tiles = {}
        steps = []  # (seg_idx, lo, n) in stream order, for the s-chain
        step_i = 0
        c_first = True
        for si_, (ts_, tn) in enumerate(segs):
            htt = htp.tile([128, tn, BS], f8, name="ht", tag="ht", bufs=NSEG)
            nc.sync.dma_start(out=htt[:], in_=ht_d[:, ts_ : ts_ + tn, :])
            ht_tiles[si_] = htt

            lo = 0
            while lo < tn:
                n = 2 if lo + 2 <= tn else 1
                t = ts_ + lo
                steps.append((si_, lo, n))
                w1s = w1p.tile([128, 2, FO1], f8, name="w1s", tag="w1s")
                if step_i == 0:
                    h = FO1 // 2
                    nc.sync.dma_start(out=w1s[:, 0:n, 0:h], in_=w1t_d[:, t : t + n, 0:h])
                    nc.sync.dma_start(out=w1s[:, 0:n, h:FO1], in_=w1t_d[:, t : t + n, h:FO1])
                else:
                    nc.sync.dma_start(out=w1s[:, 0:n, :], in_=w1t_d[:, t : t + n, :])
                if FLIP:
                    # stationary = history batch-group, moving = w1 fo-half:
                    # 4 LDWEIGHTS + 8 matmuls per k-pair (vs 8 + 8), PSUM
                    # holds x1 transposed [batch-group, fo].
                    for bg in range(4):
                        bsl = slice(bg * 128, (bg + 1) * 128)
                        for h in range(2):
                            fsl = slice(h * 512, (h + 1) * 512)
                            if n == 2:
                                nc.tensor.matmul(
                                    ps1[bg][:, h, :],
                                    lhsT=htt[:, lo : lo + 2, bsl],
                                    rhs=w1s[:, 0:2, fsl],
                                    start=(step_i == 0),
                                    stop=(step_i == nstep - 1),
                                    perf_mode=mybir.MatmulPerfMode.DoubleRow,
                                )
                            else:
                                nc.tensor.matmul(
                                    ps1[bg][:, h, :],
                                    lhsT=htt[:, lo, bsl],
                                    rhs=w1s[:, 0, fsl],
                                    start=(step_i == 0),
                                    stop=(step_i == nstep - 1),
                                )
                else:
                    for fo in range(FO1T):
                        fsl = slice(fo * 128, (fo + 1) * 128)
                        pdst = ps1[fo // 2][:, fo % 2, :]
                        if n == 2:
                            nc.tensor.matmul(
                                pdst,
                                lhsT=w1s[:, 0:2, fsl],
                                rhs=htt[:, lo : lo + 2, :],
                                start=(step_i == 0),
                                stop=(step_i == nstep - 1),
                                perf_mode=mybir.MatmulPerfMode.DoubleRow,
                            )
                        else:
                            nc.tensor.matmul(
                                pdst,
                                lhsT=w1s[:, 0, fsl],
                                rhs=htt[:, lo, :],
                                start=(step_i == 0),
                                stop=(step_i == nstep - 1),
                            )
                step_i += 1
                lo += n

            # cnt accumulation on the (otherwise idle) DVE: fused min+add,
            # paired tiles.  Segment 0 holds extInfo rows; init via masked min.
            if c_first:
                assert tn == 2
                nc.vector.tensor_scalar(
                    c_acc[:, 0, :], htt[:, 0, :], 1.0, rmask[:], op0=ALU.min, op1=ALU.mult
                )
                nc.vector.tensor_scalar(
                    c_acc[:, 1, :], htt[:, 1, :], 1.0, None, op0=ALU.min
                )
                c_first = False
            else:
                o = 0
                while o < tn:
                    if o + 2 <= tn:
                        nc.vector.scalar_tensor_tensor(
                            c_acc[:], htt[:, o : o + 2, :], 1.0, c_acc[:],
                            op0=ALU.min, op1=ALU.add,
                        )
                        o += 2
                    else:
                        nc.vector.scalar_tensor_tensor(
                            c_acc[:, 0, :], htt[:, o, :], 1.0, c_acc[:, 0, :],
                            op0=ALU.min, op1=ALU.add,
                        )
                        o += 1

        # ---- s: PE ones-chain over the resident history, double-buffered
        # across the two banks of one PSUM slot.  Emission is chunked into
        # the layer-boundary gaps (where the next layer's matmuls block on
        # epilogue ACTs anyway), so the chain fills PE idle time instead of
        # serializing before or after the mid layers.
        scx = psp.tile([16, 2, BS], f32, name="scx", tag="ps")
        NS = len(steps)
        s_pos = [0]

        def emit_s(k):
            i0 = s_pos[0]
            i1 = min(i0 + k, NS)
            for si in range(i0, i1):
                sg, lo, n = steps[si]
                htt = ht_tiles[sg]
                if sg == 0 and lo == 0:
                    ones = ones16m
                elif si == NS - 1:
                    ones = ones_last
                else:
                    ones = ones16
                dst = scx[0:16, si % 2, :]
                if n == 2:
                    nc.tensor.matmul(
                        dst, lhsT=ones[:], rhs=htt[:, lo : lo + 2, :],
                        start=(si < 2), stop=(si >= NS - 2),
                        perf_mode=mybir.MatmulPerfMode.DoubleRow,
                    )
                else:
                    nc.tensor.matmul(
                        dst, lhsT=ones[:, 0, :], rhs=htt[:, lo, :],
                        start=(si < 2), stop=(si >= NS - 2),
                    )
            s_pos[0] = i1

        # layer-1 epilogue (bias-free: bias rode the matmul via the ones-row)
        if FLIP:
            # drain [batch, fo] pre-acts to bf16, PE-transpose 128x128 blocks
            # back to [fo, batch], then fused Lrelu into fp8 x1t.
            z1b = actp.tile([128, 4, FO1], bf16, name="z1b", tag="z1b")
            for bg in range(4):
                nc.scalar.activation(
                    z1b[:, bg, :], ps1[bg][:].opt(), AF.Copy, scale=w1_unscale
                )
            for ft in range(FO1T):
                ztp = psp.tile([128, 4, 128], bf16, name="ztp", tag="ps")
                for bg in range(4):
                    nc.tensor.transpose(
                        ztp[:, bg, :], z1b[:, bg, ft * 128 : (ft + 1) * 128], ident_sb[:]
                    )
                nc.scalar.activation(
                    x1t[:, ft, :], ztp[:].opt(), AF.Lrelu, scale=1.0, alpha=SLOPE
                )
        else:
            for j in range(FO1T // 2):
                nc.scalar.activation(
                    x1t[:, 2 * j : 2 * j + 2, :], ps1[j][:], AF.Lrelu,
                    scale=w1_unscale, alpha=SLOPE,
                )

        # ---- remaining weights + movie factors: emitted late on the Sync
        # queue so the layer-1 ht/w1 stream gets the DMA bandwidth first.
        w_sb = {}
        for li in range(2, 7):
            fi, fo = DIMS[li - 2], DIMS[li - 1]
            wdt = f8 if li < 6 else bf16
            w_sb[li] = const.tile([128, fi // 128, fo], wdt, name=f"w{li}t", tag=f"w{li}t")
            nc.sync.dma_start(out=w_sb[li][:], in_=w_d[li][:])
        mft = const.tile([128, M], bf16, name="mft", tag="mft")
        nc.sync.dma_start(out=mft[:], in_=mft_d[:])
        emit_s(18)

        # ---- layers 2..5 (lrelu) ----
        xin = x1t
        for li in range(2, 6):
            fi, fo = DIMS[li - 2], DIMS[li - 1]
            fit, fot = fi // 128, fo // 128
            xdt = f8 if li < 5 else bf16
            unsc = 1.0 / cfg["MLP_SCALES"][li - 2]
            xout = actp.tile(
                [128, fot, BS], xdt, name=f"x{li}t",
                tag=("x1t" if li == 5 else "x2t" if li == 4 else f"x{li}t"),
            )
            # interleave the k-accumulation across up to 3 fo-tile banks so
            # consecutive matmuls don't serialize on one bank's drain (3,
            # not 4: the s-chain accumulator scx parks in the 4th ring slot)
            for g0 in range(0, fot, 3):
                gn = min(3, fot - g0)
                pss = [psp.tile([128, BS], f32, name="ps", tag="ps") for _ in range(gn)]
                ki = 0
                while ki < fit:
                    n2 = 2 if ki + 2 <= fit else 1
                    for j in range(gn):
                        ft = g0 + j
                        if n2 == 2:
                            nc.tensor.matmul(
                                pss[j][:],
                                lhsT=w_sb[li][:, ki : ki + 2, ft * 128 : (ft + 1) * 128],
                                rhs=xin[:, ki : ki + 2, :],
                                start=(ki == 0),
                                stop=(ki + 2 == fit),
                                perf_mode=mybir.MatmulPerfMode.DoubleRow,
                            )
                        else:
                            nc.tensor.matmul(
                                pss[j][:],
                                lhsT=w_sb[li][:, ki, ft * 128 : (ft + 1) * 128],
                                rhs=xin[:, ki, :],
                                start=(ki == 0),
                                stop=True,
                            )
                    ki += n2
                for j in range(gn):
                    ft = g0 + j
                    if li >= 4 and fot >= 4 and j == 2:
                        # offload some lrelu epilogues to the DVE (2-op form:
                        # z = ps*unsc + b; x = max(z*slope, z))
                        tmp = scr.tile([128, BS], f32, name="tmp", tag="tmp")
                        nc.vector.tensor_scalar(
                            tmp[:], pss[j][:], unsc, bp_sb[li][:, ft : ft + 1],
                            op0=ALU.mult, op1=ALU.add,
                        )
                        nc.vector.scalar_tensor_tensor(
                            xout[:, ft, :], tmp[:], SLOPE, tmp[:],
                            op0=ALU.mult, op1=ALU.max,
                        )
                    else:
                        nc.scalar.activation(
                            xout[:, ft, :], pss[j][:], AF.Lrelu,
                            bias=bp_sb[li][:, ft : ft + 1], scale=unsc, alpha=SLOPE,
                        )
            emit_s(16)
            xin = xout

        # genext allocated early: its memsets depend on nothing, so they run
        # while the DVE is otherwise idle instead of queueing ahead of the
        # addv chain later.
        genext = actp.tile([128, BS], bf16, name="genext", tag="genext")
        nc.vector.memset(genext[F : 128, :], 0.0)
        nc.vector.memset(genext[F : F + 1, :], 1.0)

        # ---- s/cnt reduces + meanV combine chain, emitted before layer 6:
        # the s-chain's last matmul lands during L5, so meanV (pre-scaled by
        # the output scale) is ready before the reconstruction needs addv.
        emit_s(NS)  # flush any remaining s-chain steps
        c_red = psp.tile([1, 2, BS], f32, name="c_red", tag="ps")
        nc.tensor.matmul(c_red[:, 0, :], lhsT=onesf[:], rhs=c_acc[:, 0, :], start=True, stop=True)
        nc.tensor.matmul(c_red[:, 1, :], lhsT=onesf[:], rhs=c_acc[:, 1, :], start=True, stop=True)
        c0_sb = const.tile([1, BS], f32, name="c0_sb", tag="c0_sb")
        nc.vector.tensor_scalar_sub(c0_sb[:], c_red[0:1, 0, :], 1.0)
        c_sb = const.tile([1, BS], f32, name="c_sb", tag="c_sb")
        nc.vector.tensor_add(c_sb[:], c0_sb[:], c_red[0:1, 1, :])
        rc_sb = const.tile([1, BS], f32, name="rc_sb", tag="rc_sb")
        nc.vector.reciprocal(rc_sb[:], c_sb[:])
        # s pre-scaled by OSC while combining the two chain halves
        s_sb = const.tile([1, BS], f32, name="s_sb", tag="s_sb")
        nc.vector.tensor_scalar(s_sb[:], scx[0:1, 0, :], float(OSC), None, op0=ALU.mult)
        nc.vector.scalar_tensor_tensor(
            s_sb[:], scx[0:1, 1, :], float(OSC), s_sb[:], op0=ALU.mult, op1=ALU.add
        )
        mv_sb = const.tile([1, BS], f32, name="mv_sb", tag="mv_sb")
        nc.vector.tensor_mul(mv_sb[:], rc_sb[:], s_sb[:])

        # ---- layer 6 (tanh) -> genf [65, BS] f32 ----
        fi, fo = DIMS[4], DIMS[5]
        fit = fi // 128
        assert fo == F + 1
        ps6 = psp.tile([fo, BS], f32, name="ps6", tag="ps")
        for ki in range(fit):
            nc.tensor.matmul(
                ps6[:],
                lhsT=w_sb[6][:, ki, 0:fo],
                rhs=xin[:, ki, :],
                start=(ki == 0),
                stop=(ki == fit - 1),
            )
        genf = actp.tile([fo, BS], f32, name="genf", tag="genf")
        nc.scalar.activation(genf[:], ps6[:], AF.Tanh, bias=b6_sb[0:fo, 0:1], scale=1.0)

        # genext factor rows copy on the Scalar engine (right after tanh on
        # the same queue), keeping the DVE free for the addv chain.
        nc.scalar.activation(genext[0:F, :], genf[0:F, :], AF.Copy)
        gl_sb = const.tile([1, BS], f32, name="gl_sb", tag="gl_sb")
        nc.sync.dma_start(out=gl_sb[:], in_=genf[F : F + 1, :])

        # addv = meanV*OSC + gen_last*OSC, bounced through DRAM into
        # per-partition layout [128, NBT] for the staging ops.
        av_sb = const.tile([1, BS], f32, name="av_sb", tag="av_sb")
        nc.vector.scalar_tensor_tensor(
            av_sb[:], gl_sb[:], float(OSC), mv_sb[:], op0=ALU.mult, op1=ALU.add
        )
        nc.sync.dma_start(out=av_d[:], in_=av_sb[0:1, :])
        addv_t = const.tile([128, NBT], f32, name="addv_t", tag="addv_t")
        nc.sync.dma_start(out=addv_t[:], in_=av_d.ap().rearrange("(t p) -> p t", p=128))

        # ---- reconstruction: out[bt*128+p, m] over movie chunk-pairs ----
        PAIRS = [CHUNKS[i : i + 2] for i in range(0, len(CHUNKS), 2)]
        for bt in range(NBT):
            lhsT = genext[:, bt * 128 : (bt + 1) * 128]
            st = None
            for pi, pair in enumerate(PAIRS):
                eng = 0 if (pi % 9) in (0, 2, 4, 6) else 1  # 11:9 Scalar:Vector
                pr = psp.tile([128, 2, 512], f32, name="pr", tag="ps")
                for j, (co, cw) in enumerate(pair):
                    nc.tensor.matmul(
                        pr[:, j, 0:cw], lhsT=lhsT, rhs=mft[:, co : co + cw],
                        start=True, stop=True,
                    )
                pw = sum(cw for _, cw in pair)
                if pi % 2 == 0:
                    st = ost.tile([128, 2048], odt, name="st", tag="st")
                    so, po = 0, pair[0][0]
                # stage the full [2,512] pair; only the valid prefix is DMA'd
                nst = 1024 if pw == 1024 else 512 + pair[1][1]
                pr2d = pr[:].opt()  # [128, 2, 512] -> contiguous [128, 1024]
                if eng == 0:
                    nc.vector.tensor_scalar(
                        st[:, so : so + 1024], pr2d, OSC, addv_t[:, bt : bt + 1],
                        op0=ALU.mult, op1=ALU.add,
                    )
                else:
                    nc.scalar.activation(
                        st[:, so : so + 1024], pr2d, AF.Identity,
                        bias=addv_t[:, bt : bt + 1], scale=OSC,
                    )
                so += nst
                if pi % 2 == 1 or pi == len(PAIRS) - 1:
                    nc.sync.dma_start(
                        out=out_d[bt * 128 : (bt + 1) * 128, po : po + so],
                        in_=st[:, 0:so],
                    )

    nc.compile()
    return nc


def prep_in_maps(cfg, inputs):
    """Shard + lay out the full inputs into per-core DRAM input maps."""
    d = _derived(cfg)
    BS, UINFO, M, F, DIMS, T1 = cfg["BS"], cfg["UINFO"], cfg["M"], cfg["F"], cfg["DIMS"], d["T1"]
    extInfo = np.asarray(inputs["extInfo"], np.float32)
    ratings = np.asarray(inputs["ratings"], np.float32)

    # BN (eval) fold into layer 2: y = g'(lrelu1) + b' with g' = bn_g/sqrt(1+eps)
    g = np.asarray(inputs["bn_g"], np.float32) / np.float32(np.sqrt(1.0 + BN_EPS))
    bnb = np.asarray(inputs["bn_b"], np.float32)
    w2 = np.asarray(inputs["w2"], np.float32)
    w2f = w2 * g[None, :]
    b2f = np.asarray(inputs["b2"], np.float32) + w2 @ bnb

    shared = {}
    # w1t: [KH,FO1] -> padded [T1*128, FO1] -> [128, T1, FO1]; the row at
    # index UINFO+M carries b1 (matching the ones-row in the history).
    w1 = np.asarray(inputs["w1"], np.float32)
    b1 = np.asarray(inputs["b1"], np.float32)
    FO1 = DIMS[0]
    w1tp = np.zeros((T1 * 128, FO1), FP8)
    w1tp[0 : w1.shape[1]] = (w1.T * np.float32(cfg["W1_SCALE"])).astype(FP8)
    w1tp[UINFO + M] = (b1 * np.float32(cfg["W1_SCALE"])).astype(FP8)
    shared["w1t"] = np.ascontiguousarray(w1tp.reshape(T1, 128, FO1).transpose(1, 0, 2))

    def pack_w(wT, fo, dt=BF16, scale=1.0):
        fi = wT.shape[0]
        w = (wT.astype(np.float32) * np.float32(scale)).astype(dt)
        return np.ascontiguousarray(w.reshape(fi // 128, 128, fo).transpose(1, 0, 2))

    scs = cfg["MLP_SCALES"]
    shared["w2t"] = pack_w(w2f.T, DIMS[1], FP8, scs[0])
    for li, wname in ((3, "w3"), (4, "w4"), (5, "w5"), (6, "w6")):
        w = np.asarray(inputs[wname], np.float32)
        fo = DIMS[li - 1]
        if li < 6:
            shared[f"w{li}t"] = pack_w(w.T, fo, FP8, scs[li - 2])
        else:
            shared[f"w{li}t"] = pack_w(w.T, fo)

    def pack_b(b, fo):
        fot = math.ceil(fo / 128)
        bp = np.zeros(fot * 128, np.float32)
        bp[:fo] = b
        return np.ascontiguousarray(bp.reshape(fot, 128).T)

    bsrc = {2: b2f}
    for li in (3, 4, 5):
        bsrc[li] = np.asarray(inputs[f"b{li}"], np.float32)
    for li in range(2, 6):
        shared[f"b{li}p"] = pack_b(bsrc[li], DIMS[li - 1])
    shared["b6p"] = pack_b(np.asarray(inputs["b6"], np.float32), DIMS[5])

    # bf16 mft: rows 0..F-1 factors, row F movie_bias, rows F+1/F+2 ones
    # (they pick up the addv hi/lo rows of genext).
    mft = np.zeros((128, M), BF16)
    mft[0:F] = np.asarray(inputs["movie_factors"], np.float32).T.astype(BF16)
    mft[F] = np.asarray(inputs["movie_bias"], np.float32).astype(BF16)
    mft[F + 1] = np.float32(1.0)
    mft[F + 2] = np.float32(1.0)
    shared["mft"] = mft
    if cfg.get("FLIP_L1"):
        shared["ident"] = np.eye(128, dtype=BF16)

    in_maps = []
    for c in range(NCORES):
        sl = slice(c * BS, (c + 1) * BS)
        htc = np.zeros((T1 * 128, BS), FP8)
        htc[0:UINFO] = extInfo[sl].T.astype(FP8)
        htc[UINFO : UINFO + M] = ratings[sl].T.astype(FP8)
        htc[UINFO + M] = np.float32(1.0)  # ones-row: picks up b1 from w1t
        m = dict(shared)
        m["ht"] = np.ascontiguousarray(htc.reshape(T1, 128, BS).transpose(1, 0, 2))
        in_maps.append(m)
    return in_maps


_NC_CACHE = {}


def run_on_hw(cfg, inputs, trace=False):
    from concourse.bass_utils import run_bass_kernel_spmd

    key = tuple(sorted((k, v) for k, v in cfg.items() if k != "DIMS")) + (cfg["DIMS"],)
    if key not in _NC_CACHE:
        _NC_CACHE[key] = build_nc(cfg)
    nc = _NC_CACHE[key]
    in_maps = prep_in_maps(cfg, inputs)
    br = run_bass_kernel_spmd(nc, in_maps, list(range(NCORES)), trace=trace)
    BS, M = cfg["BS"], cfg["M"]
    out = np.empty((NCORES * BS, M), np.float32)
    dq = np.float32(1.0 / cfg["OUT_SCALE"]) if cfg["OUT_DT"] == "i8" else np.float32(1.0)
    for c in range(NCORES):
        out[c * BS : (c + 1) * BS] = np.asarray(br.results[c]["out"], dtype=np.float32) * dq
    return out, br


def kernel(**inputs) -> np.ndarray:
    try:
        out, _ = run_on_hw(FULL_CFG, inputs, trace=False)
    except Exception:
        # one retry for transient device/runtime hiccups
        out, _ = run_on_hw(FULL_CFG, inputs, trace=False)
    return out
